# revision 22
# baseline (speedup 1.0000x reference)
"""Multi-head attention (b=2, t=2048, k=1024, 16 heads) on 8 TRN2 NeuronCores.

Sharding: batch across 2 groups of 4 cores; within a group, heads are
tensor-parallel (4 heads/core = 2 head-pairs, full T).  Per-core pipeline:

  head:    Q/K proj for head-pair 0 (k-outer, 8 psum banks) + V for hp0.
  phase 2: per (hp, q4-chunk, km): S^T matmuls -> exp [128,1024] on ACT.
           O matmuls use V tiles with 64 ones-columns appended, so each O
           accumulation also produces the softmax denominators replicated in
           psum rows 64-127 (no separate denominator matmuls).  The ACT
           engine is the bottleneck; PE slack is filled with interleaved
           hp1 projections (during hp0 attention) and the Wo k0-3 partial
           pass (during hp1 attention, after hp0's AllToAll).
           Normalize: evacuate op psum (DVE+ACT copies), reciprocal_approx,
           stream_shuffle partition realign, multiply, DMA to DRAM.
  comms:   one 4-core AllToAll per head-pair (each peer gets only its own
           512 token columns) instead of AllGather.
  tail:    Wo k4-7 matmuls + add to k0-3 partials -> y^T slice (bf16).
"""

import sys

if '/opt/trn_rl_repo' not in sys.path:
    sys.path.insert(0, '/opt/trn_rl_repo')

import ml_dtypes
import numpy as np

B = 2
T = 2048
KD = 1024
NH = 16
HS = 64
NCORES = 8
GROUP = 4                 # cores per batch group
NH_LOC = NH // GROUP      # heads per core
NHP = NH_LOC // 2         # head-pairs per core
TSLICE = T // GROUP       # output tokens per core
HFEAT = NH_LOC * HS       # 256 local head features
NKT = T // 128            # 16 key-token tiles
NKD = KD // 128           # 8 model-feature tiles
NQ4 = T // 512            # 4 query chunks of 512

_CACHE = {}


def _build():
    import concourse.bass as bass
    import concourse.mybir as mybir
    import concourse.tile as tile
    from concourse import bacc

    F32 = mybir.dt.float32
    BF16 = mybir.dt.bfloat16
    AF = mybir.ActivationFunctionType
    IDENT32 = list(range(32))

    nc = bacc.Bacc("TRN2", target_bir_lowering=False, debug=False,
                   num_devices=NCORES)

    xT = nc.dram_tensor("xT", [KD, T], BF16, kind="ExternalInput")
    wqT = nc.dram_tensor("wqT", [KD, HFEAT], BF16, kind="ExternalInput")
    wkT = nc.dram_tensor("wkT", [KD, HFEAT], BF16, kind="ExternalInput")
    wvT = nc.dram_tensor("wvT", [KD, HFEAT], BF16, kind="ExternalInput")
    woT = nc.dram_tensor("woT", [KD, KD], BF16, kind="ExternalInput")
    yTb = nc.dram_tensor("yTb", [KD, TSLICE], BF16, kind="ExternalOutput")

    rgroups = [list(range(GROUP)), list(range(GROUP, 2 * GROUP))]

    with tile.TileContext(nc) as tc:
        with (
            tc.tile_pool(name="qk", bufs=1) as qk_pool,
            tc.tile_pool(name="vpp", bufs=1) as vp_pool,
            tc.tile_pool(name="xw", bufs=1) as xw_pool,
            tc.tile_pool(name="wop", bufs=1) as wo_pool,
            tc.tile_pool(name="pt", bufs=7) as pt_pool,
            tc.tile_pool(name="oc", bufs=2) as oc_pool,
            tc.tile_pool(name="rb", bufs=2) as rb_pool,
            tc.tile_pool(name="on", bufs=2) as on_pool,
            tc.tile_pool(name="rhs", bufs=1) as rhs_pool,
            tc.tile_pool(name="ya", bufs=1) as ya_pool,
            tc.tile_pool(name="yt", bufs=2) as yt_pool,
            tc.tile_pool(name="dram", bufs=1, space="DRAM") as dram,
        ):
            qt = [qk_pool.tile([128, T], BF16, name=f"qt{m}", tag=f"qt{m}")
                  for m in range(2)]
            kt = [qk_pool.tile([128, T], BF16, name=f"kt{m}", tag=f"kt{m}")
                  for m in range(2)]
            # V in bf16 token-major; per head 128 lhs columns: 0-63 = V dims,
            # 64-127 = 1.0 so the O matmul also emits denominator rows.
            vp = vp_pool.tile([128, NKT, NH_LOC, 128], BF16)
            nc.vector.memset(vp[:], 1.0)

            wq = xw_pool.tile([128, NKD, HFEAT], BF16)
            wk = xw_pool.tile([128, NKD, HFEAT], BF16)
            wv = xw_pool.tile([128, NKD, HFEAT], BF16)
            xt = xw_pool.tile([128, NKD, T], BF16)
            wo = wo_pool.tile([128, NKD, KD], BF16)

            agin = [dram.tile([2 * GROUP, 128, TSLICE], BF16, name=f"agin{h}",
                              tag=f"agin{h}") for h in range(NHP)]
            agout = [dram.tile([2 * GROUP, 128, TSLICE], BF16,
                               name=f"agout{h}", tag=f"agout{h}")
                     for h in range(NHP)]

            # x slices stream on the SP queue; batched weight loads ride the
            # ACT queue so they don't serialize behind the x stream
            for k in range(NKD):
                nc.sync.dma_start(xt[:, k, :], xT.ap()[128 * k:128 * (k + 1), :])
            for wtile, wdram in ((wq, wqT), (wk, wkT), (wv, wvT)):
                nc.scalar.dma_start(
                    wtile[:], wdram.ap().rearrange("(k p) t -> p k t", p=128))

            # zero-fill the cross-group AllToAll blocks (the other batch
            # group never reads them, but they must be defined)
            zt = vp_pool.tile([128, 2 * GROUP * TSLICE], BF16, name="zt")
            nc.vector.memset(zt[:], 0.0)
            for h in range(NHP):
                nc.sync.dma_start(
                    agin[h][:, :, :].rearrange("j p t -> p j t"),
                    zt[:].rearrange("p (j t) -> p j t", j=2 * GROUP))

            # ---- head: Q/K proj for hp0 (m=0), then V for hp0 ----
            with tc.tile_pool(name="hpsum", bufs=1, space="PSUM") as hpsum:
                acc = [hpsum.tile([128, 512], F32, name=f"acc{i}", tag=f"acc{i}")
                       for i in range(8)]
                for k in range(NKD):
                    for n in range(4):
                        nc.tensor.matmul(
                            acc[n][:], wq[:, k, 0:128],
                            xt[:, k, 512 * n:512 * (n + 1)],
                            start=(k == 0), stop=(k == NKD - 1))
                    for n in range(4):
                        nc.tensor.matmul(
                            acc[4 + n][:], wk[:, k, 0:128],
                            xt[:, k, 512 * n:512 * (n + 1)],
                            start=(k == 0), stop=(k == NKD - 1))
                for n in range(4):
                    nc.vector.tensor_copy(
                        qt[0][:, 512 * n:512 * (n + 1)], acc[n][:])
                    nc.vector.tensor_copy(
                        kt[0][:, 512 * n:512 * (n + 1)], acc[4 + n][:])

                vcopy = None
                for mt in range(NKT):
                    ps = hpsum.tile([128, 128], F32, tag=f"acc{mt % 8}")
                    for k in range(NKD):
                        nc.tensor.matmul(
                            ps[:], xt[:, k, 128 * mt:128 * (mt + 1)],
                            wv[:, k, 0:128],
                            start=(k == 0), stop=(k == NKD - 1))
                    vcopy = nc.vector.tensor_copy(
                        vp[:, mt, 0:2, 0:64],
                        ps[:].rearrange("p (h d) -> p h d", h=2))

            # wo prefetch, deferred until the head V copies so it doesn't
            # steal HBM bandwidth from the x/weight loads
            wdma = nc.sync.dma_start(
                wo[:], woT.ap().rearrange("(k p) t -> p k t", p=128))
            tile.add_dep_helper(vcopy.ins, wdma.ins, sync=False,
                                reason="defer wo prefetch past head proj")

            # ---- phase 2: attention, ACT-bound, PE slack filled with
            #      hp1 projections and the wo k0-3 partial pass ----
            with (
                tc.tile_pool(name="spsum", bufs=2, space="PSUM") as spsum,
                tc.tile_pool(name="opsum", bufs=1, space="PSUM") as opsum,
                tc.tile_pool(name="auxsum", bufs=2, space="PSUM") as auxsum,
            ):
                pid = nc.partition_id()
                rnk = pid & 3          # rank within the 4-core batch group
                gb4 = pid & 4          # first core of my batch group

                # ---- filler work for hp0 attention slots: Q1/K1/V1 proj ----
                fill0 = []  # (cols, closure)
                _auxn = [0]

                def qk1_unit(wtile, dst, n):
                    def mmk(k, wtile=wtile, n=n, start=False, stop=False):
                        def fn(aux):
                            nc.tensor.matmul(
                                aux[:], wtile[:, k, 128:256],
                                xt[:, k, 512 * n:512 * (n + 1)],
                                start=(k == 0), stop=(k == NKD - 1))
                        return fn
                    def cpy(aux, dst=dst, n=n):
                        nc.vector.tensor_copy(
                            dst[:, 512 * n:512 * (n + 1)], aux[:])
                    return [mmk(k) for k in range(NKD)], cpy, [128, 512]

                def v1_unit(mt):
                    def mmk(k, mt=mt):
                        def fn(aux):
                            nc.tensor.matmul(
                                aux[:], xt[:, k, 128 * mt:128 * (mt + 1)],
                                wv[:, k, 128:256],
                                start=(k == 0), stop=(k == NKD - 1))
                        return fn
                    def cpy(aux, mt=mt):
                        nc.vector.tensor_copy(
                            vp[:, mt, 2:4, 0:64],
                            aux[:].rearrange("p (h d) -> p h d", h=2))
                    return [mmk(k) for k in range(NKD)], cpy, [128, 128]

                units = ([qk1_unit(wk, kt[1], n) for n in range(4)]
                         + [qk1_unit(wq, qt[1], 0)]
                         + [v1_unit(mt) for mt in range(NKT)]
                         + [qk1_unit(wq, qt[1], n) for n in range(1, 4)])
                for mms, cpy, shape in units:
                    aux_box = []
                    def alloc(aux_box=aux_box, shape=shape):
                        aux_box.append(auxsum.tile(shape, F32, name=f"aux{_auxn[0]}", tag="aux"))
                        _auxn[0] += 1
                    cols = shape[1]
                    for i, mm in enumerate(mms):
                        def step(mm=mm, aux_box=aux_box, alloc=alloc, first=(i == 0)):
                            if first:
                                alloc()
                            mm(aux_box[-1])
                        fill0.append((cols, step))
                    fill0.append((0, lambda cpy=cpy, aux_box=aux_box:
                                  cpy(aux_box[-1])))

                # ---- filler work for hp1 attention slots: wo k0-3 pass ----
                rhs = [rhs_pool.tile([128, TSLICE], BF16, name=f"rhs{k}",
                                     tag=f"rhs{k}") for k in range(NKD)]
                ya = [ya_pool.tile([128, TSLICE], F32, name=f"ya{m}",
                                   tag=f"ya{m}") for m in range(NKD)]
                fill1 = []

                def woa_unit(m):
                    aux_box = []
                    def step(k, m=m, aux_box=aux_box):
                        def fn():
                            if k == 0:
                                aux_box.append(
                                    auxsum.tile([128, TSLICE], F32, name=f"auxw{m}", tag="aux"))
                            nc.tensor.matmul(
                                aux_box[-1][:], wo[:, k, 128 * m:128 * (m + 1)],
                                rhs[k][:], start=(k == 0), stop=(k == 3))
                        return fn
                    def cpy(m=m, aux_box=aux_box):
                        nc.vector.tensor_copy(ya[m][:], aux_box[-1][:])
                    return [step(k) for k in range(4)], cpy

                for m in range(NKD):
                    mms, cpy = woa_unit(m)
                    for mm in mms:
                        fill1.append((512, mm))
                    fill1.append((0, cpy))

                fstate = {"i0": 0, "c0": 0.0, "i1": 0, "c1": 0.0}

                def pump0(target):
                    while fstate["i0"] < len(fill0) and fstate["c0"] < target:
                        cols, fn = fill0[fstate["i0"]]
                        fstate["i0"] += 1
                        fn()
                        fstate["c0"] += cols

                def pump1(target):
                    while fstate["i1"] < len(fill1) and fstate["c1"] < target:
                        cols, fn = fill1[fstate["i1"]]
                        fstate["i1"] += 1
                        fn()
                        fstate["c1"] += cols

                WOA_START = 96   # slot where the wo k0-3 pass may begin

                def pump(slot):
                    if slot < 64:
                        pump0(640.0 * (slot + 1))
                    else:
                        pump0(40960.0 + 512.0 * (slot - 63))
                        if slot >= WOA_START:
                            pump1(768.0 * (slot - WOA_START + 1))

                # ---- attention slot loop ----
                pend = {}

                def add_pend(slot, fn):
                    pend.setdefault(slot, []).append(fn)

                def make_oslot(km, opA, opB, pts, hA, hB):
                    def fn():
                        nc.tensor.matmul(
                            opA[:], vp[:, km, hA, :], pts[km][:, 0:512],
                            start=(km == 0), stop=(km == NKT - 1))
                        nc.tensor.matmul(
                            opB[:], vp[:, km, hB, :], pts[km][:, 512:1024],
                            start=(km == 0), stop=(km == NKT - 1))
                    return fn

                def make_finalize(hp, q4, opA, opB):
                    def fn():
                        # evacuate psum on two engines in parallel
                        ocA = oc_pool.tile([128, 512], F32, tag="ocA")
                        ocB = oc_pool.tile([128, 512], F32, tag="ocB")
                        nc.vector.tensor_copy(ocA[:], opA[:])
                        nc.scalar.copy(ocB[:], opB[:])
                        # stack both heads' denominators -> one reciprocal
                        da = rb_pool.tile([128, 512], F32, tag="da")
                        nc.vector.stream_shuffle(
                            da[0:64, :], ocA[64:128, :], IDENT32)
                        nc.vector.tensor_copy(da[64:128, :], ocB[64:128, :])
                        rr = rb_pool.tile([128, 512], F32, tag="rr")
                        nc.vector.reciprocal(rr[:], da[:])
                        onA = on_pool.tile([64, 512], BF16, tag="onA")
                        onB = on_pool.tile([64, 512], BF16, tag="onB")
                        nc.vector.tensor_mul(onA[:], ocA[0:64, :], rr[0:64, :])
                        rlB = rb_pool.tile([64, 512], F32, tag="rlB")
                        nc.vector.stream_shuffle(rlB[:], rr[64:128, :], IDENT32)
                        nc.vector.tensor_mul(onB[:], ocB[0:64, :], rlB[:])
                        nc.sync.dma_start(
                            agin[hp][bass.ds(gb4 + q4, 1), 0:64, :].squeeze(0),
                            onA[:])
                        nc.sync.dma_start(
                            agin[hp][bass.ds(gb4 + q4, 1), 64:128, :].squeeze(0),
                            onB[:])
                        if q4 == NQ4 - 1:
                            nc.gpsimd.collective_compute(
                                "AllToAll",
                                mybir.AluOpType.bypass,
                                replica_groups=[list(range(NCORES))],
                                ins=[agin[hp].opt()],
                                outs=[agout[hp].opt()],
                            )
                    return fn

                slot = 0
                for hp in range(NHP):
                    hA, hB = 2 * hp, 2 * hp + 1
                    for q4 in range(NQ4):
                        c = hp * NQ4 + q4
                        base = 16 * c
                        opA = opsum.tile([128, 512], F32, tag="opA")
                        opB = opsum.tile([128, 512], F32, tag="opB")
                        pts = {}
                        oslot = [make_oslot(km, opA, opB, pts, hA, hB)
                                 for km in range(NKT)]
                        for km in range(NKT - 2):
                            add_pend(base + km + 3, oslot[km])
                        fin = make_finalize(hp, q4, opA, opB)
                        add_pend(base + 17,
                                 lambda a=oslot[14], b=oslot[15], f=fin:
                                 (a(), b(), f()))
                        if c == 3:
                            # prefetch wo rhs for hp0 once its AllToAll is done
                            add_pend(base + 17, lambda: [
                                nc.sync.dma_start(
                                    rhs[k][:],
                                    agout[0][bass.ds(gb4 + k % GROUP, 1),
                                             :, :].squeeze(0))
                                for k in range(GROUP)])

                        qs = slice(512 * q4, 512 * (q4 + 1))
                        for km in range(NKT):
                            ks = slice(128 * km, 128 * (km + 1))
                            sp = spsum.tile([128, 1024], F32, tag="sp")
                            nc.tensor.matmul(
                                sp[:, 0:512], kt[hp][0:64, ks],
                                qt[hp][0:64, qs], start=True, stop=True)
                            nc.tensor.matmul(
                                sp[:, 512:1024], kt[hp][64:128, ks],
                                qt[hp][64:128, qs], start=True, stop=True)
                            pt = pt_pool.tile([128, 1024], BF16, tag="pt")
                            nc.scalar.activation(pt[:], sp[:], AF.Exp,
                                                 scale=0.03125)
                            pts[km] = pt
                            for fn in pend.pop(slot, []):
                                fn()
                            pump(slot)
                            slot += 1

                # flush trailing scheduled items (last chunk's O + finalize)
                for s in sorted(pend):
                    for fn in pend.pop(s):
                        fn()
                pump0(1e18)
                pump1(1e18)

                # ---- tail: rhs for hp1, wo k4-7 + add partials, output ----
                for k in range(GROUP, NKD):
                    nc.sync.dma_start(
                        rhs[k][:],
                        agout[1][bass.ds(gb4 + k % GROUP, 1), :, :].squeeze(0))
                for m in range(NKD):
                    yb = auxsum.tile([128, TSLICE], F32, name=f"yb{m}", tag="aux")
                    for k in range(GROUP, NKD):
                        nc.tensor.matmul(
                            yb[:], wo[:, k, 128 * m:128 * (m + 1)], rhs[k][:],
                            start=(k == GROUP), stop=(k == NKD - 1))
                    yt_s = yt_pool.tile([128, TSLICE], BF16, tag="yt")
                    nc.vector.tensor_add(yt_s[:], yb[:], ya[m][:])
                    nc.sync.dma_start(yTb.ap()[128 * m:128 * (m + 1), :], yt_s[:])

    nc.compile()
    return nc


def _get_nc():
    if "nc" not in _CACHE:
        _CACHE["nc"] = _build()
    return _CACHE["nc"]


def _make_in_maps(x, Wq, Wk, Wv, Wo):
    # Wo rows permuted to match the AllToAll assembly order:
    # row i = (pair hp, source rank s, head-in-pair a, dim d) -> head 4s+2hp+a
    idx = np.arange(KD)
    hp, rem = idx // (GROUP * 128), idx % (GROUP * 128)
    s, r = rem // 128, rem % 128
    a, d = r // HS, r % HS
    perm = (GROUP * s + 2 * hp + a) * HS + d
    woTp = np.ascontiguousarray(Wo.T[perm]).astype(ml_dtypes.bfloat16)

    in_maps = []
    for c in range(NCORES):
        g, r = c // GROUP, c % GROUP
        rows = slice(r * HFEAT, (r + 1) * HFEAT)
        in_maps.append({
            "xT": np.ascontiguousarray(x[g].T).astype(ml_dtypes.bfloat16),
            "wqT": np.ascontiguousarray(Wq[rows].T).astype(ml_dtypes.bfloat16),
            "wkT": np.ascontiguousarray(Wk[rows].T).astype(ml_dtypes.bfloat16),
            "wvT": np.ascontiguousarray(Wv[rows].T).astype(ml_dtypes.bfloat16),
            "woT": woTp,
        })
    return in_maps


def kernel(x, Wq, Wk, Wv, Wo):
    from concourse import bass_utils

    x = np.asarray(x, dtype=np.float32)
    Wq = np.asarray(Wq, dtype=np.float32)
    Wk = np.asarray(Wk, dtype=np.float32)
    Wv = np.asarray(Wv, dtype=np.float32)
    Wo = np.asarray(Wo, dtype=np.float32)

    nc = _get_nc()
    in_maps = _make_in_maps(x, Wq, Wk, Wv, Wo)
    res = bass_utils.run_bass_kernel_spmd(nc, in_maps, core_ids=list(range(NCORES)))

    out = np.empty((B, T, KD), dtype=np.float32)
    for c in range(NCORES):
        g, r = c // GROUP, c % GROUP
        out[g, r * TSLICE:(r + 1) * TSLICE, :] = \
            res.results[c]["yTb"].astype(np.float32).T
    return out


# revision 32
# speedup vs baseline: 1.0471x; 1.0471x over previous
"""Multi-head attention (b=2, t=2048, k=1024, 16 heads) on 8 TRN2 NeuronCores.

Sharding: batch across 2 groups of 4 cores; within a group, heads are
tensor-parallel (4 heads/core = 2 head-pairs, full T).  Per-core pipeline:

  head:    Q/K proj for head-pair 0 (k-outer, 8 psum banks) + V for hp0.
  phase 2: per (hp, q4-chunk, km): S^T matmuls -> exp [128,1024] on ACT.
           O matmuls use V tiles with 64 ones-columns appended, so each O
           accumulation also produces the softmax denominators replicated in
           psum rows 64-127 (no separate denominator matmuls).  The ACT
           engine is the bottleneck; PE slack is filled with interleaved
           hp1 projections (during hp0 attention) and the Wo k0-3 partial
           pass (during hp1 attention, after hp0's AllToAll).
           Normalize: evacuate op psum (DVE+ACT copies), reciprocal_approx,
           stream_shuffle partition realign, multiply, DMA to DRAM.
  comms:   one 4-core AllToAll per head-pair (each peer gets only its own
           512 token columns) instead of AllGather.
  tail:    Wo k4-7 matmuls + add to k0-3 partials -> y^T slice (bf16).
"""

import sys

if '/opt/trn_rl_repo' not in sys.path:
    sys.path.insert(0, '/opt/trn_rl_repo')

import ml_dtypes
import numpy as np

B = 2
T = 2048
KD = 1024
NH = 16
HS = 64
NCORES = 8
GROUP = 4                 # cores per batch group
NH_LOC = NH // GROUP      # heads per core
NHP = NH_LOC // 2         # head-pairs per core
TSLICE = T // GROUP       # output tokens per core
HFEAT = NH_LOC * HS       # 256 local head features
NKT = T // 128            # 16 key-token tiles
NKD = KD // 128           # 8 model-feature tiles
NQ4 = T // 512            # 4 query chunks of 512

_CACHE = {}


def _build():
    import concourse.bass as bass
    import concourse.mybir as mybir
    import concourse.tile as tile
    from concourse import bacc

    F32 = mybir.dt.float32
    BF16 = mybir.dt.bfloat16
    AF = mybir.ActivationFunctionType
    IDENT32 = list(range(32))

    nc = bacc.Bacc("TRN2", target_bir_lowering=False, debug=False,
                   num_devices=NCORES)

    xT = nc.dram_tensor("xT", [KD, T], BF16, kind="ExternalInput")
    wqT = nc.dram_tensor("wqT", [KD, HFEAT], BF16, kind="ExternalInput")
    wkT = nc.dram_tensor("wkT", [KD, HFEAT], BF16, kind="ExternalInput")
    wvT = nc.dram_tensor("wvT", [KD, HFEAT], BF16, kind="ExternalInput")
    woT = nc.dram_tensor("woT", [KD, KD], BF16, kind="ExternalInput")
    yTb = nc.dram_tensor("yTb", [KD, TSLICE], BF16, kind="ExternalOutput")

    rgroups = [list(range(GROUP)), list(range(GROUP, 2 * GROUP))]

    with tile.TileContext(nc) as tc:
        with (
            tc.tile_pool(name="qk", bufs=1) as qk_pool,
            tc.tile_pool(name="vpp", bufs=1) as vp_pool,
            tc.tile_pool(name="xw", bufs=1) as xw_pool,
            tc.tile_pool(name="wop", bufs=1) as wo_pool,
            tc.tile_pool(name="pt", bufs=7) as pt_pool,
            tc.tile_pool(name="oc", bufs=2) as oc_pool,
            tc.tile_pool(name="rb", bufs=2) as rb_pool,
            tc.tile_pool(name="on", bufs=2) as on_pool,
            tc.tile_pool(name="rhs", bufs=1) as rhs_pool,
            tc.tile_pool(name="ya", bufs=1) as ya_pool,
            tc.tile_pool(name="yt", bufs=2) as yt_pool,
            tc.tile_pool(name="dram", bufs=1, space="DRAM") as dram,
        ):
            qt = [qk_pool.tile([128, T], BF16, name=f"qt{m}", tag=f"qt{m}")
                  for m in range(2)]
            kt = [qk_pool.tile([128, T], BF16, name=f"kt{m}", tag=f"kt{m}")
                  for m in range(2)]
            # V in bf16 token-major; per head 128 lhs columns: 0-63 = V dims,
            # 64-127 = 1.0 so the O matmul also emits denominator rows.
            vp = vp_pool.tile([128, NKT, NH_LOC, 128], BF16)
            nc.vector.memset(vp[:], 1.0)

            wq = xw_pool.tile([128, NKD, HFEAT], BF16)
            wk = xw_pool.tile([128, NKD, HFEAT], BF16)
            wv = xw_pool.tile([128, NKD, HFEAT], BF16)
            xt = xw_pool.tile([128, NKD, T], BF16)
            wo = wo_pool.tile([128, NKD, KD], BF16)

            # hp0 gathers at query-half granularity, hp1 at q4 granularity
            # (small final op keeps the collective off the critical tail)
            agin0 = dram.tile([2, 128, 2 * TSLICE], BF16, name="agin0")
            agout0 = dram.tile([2, GROUP, 128, 2 * TSLICE], BF16, name="agout0")
            agin1 = dram.tile([NQ4, 128, TSLICE], BF16, name="agin1")
            agout1 = dram.tile([NQ4, GROUP, 128, TSLICE], BF16, name="agout1")

            # x slices stream on the SP queue; batched weight loads ride the
            # ACT queue so they don't serialize behind the x stream
            for k in range(NKD):
                nc.sync.dma_start(xt[:, k, :], xT.ap()[128 * k:128 * (k + 1), :])
            for wtile, wdram in ((wq, wqT), (wk, wkT), (wv, wvT)):
                nc.scalar.dma_start(
                    wtile[:], wdram.ap().rearrange("(k p) t -> p k t", p=128))



            # ---- head: Q/K proj for hp0 (m=0), then V for hp0 ----
            with tc.tile_pool(name="hpsum", bufs=1, space="PSUM") as hpsum:
                acc = [hpsum.tile([128, 512], F32, name=f"acc{i}", tag=f"acc{i}")
                       for i in range(8)]
                for k in range(NKD):
                    for n in range(4):
                        nc.tensor.matmul(
                            acc[n][:], wq[:, k, 0:128],
                            xt[:, k, 512 * n:512 * (n + 1)],
                            start=(k == 0), stop=(k == NKD - 1))
                    for n in range(4):
                        nc.tensor.matmul(
                            acc[4 + n][:], wk[:, k, 0:128],
                            xt[:, k, 512 * n:512 * (n + 1)],
                            start=(k == 0), stop=(k == NKD - 1))
                for n in range(4):
                    nc.vector.tensor_copy(
                        qt[0][:, 512 * n:512 * (n + 1)], acc[n][:])
                    nc.vector.tensor_copy(
                        kt[0][:, 512 * n:512 * (n + 1)], acc[4 + n][:])

                vcopy = None
                for mt in range(NKT // 2):
                    ps = hpsum.tile([128, 128], F32, tag=f"acc{mt % 8}")
                    for k in range(NKD):
                        nc.tensor.matmul(
                            ps[:], xt[:, k, 128 * mt:128 * (mt + 1)],
                            wv[:, k, 0:128],
                            start=(k == 0), stop=(k == NKD - 1))
                    vcopy = nc.vector.tensor_copy(
                        vp[:, mt, 0:2, 0:64],
                        ps[:].rearrange("p (h d) -> p h d", h=2))

            # wo prefetch, deferred until the head V copies so it doesn't
            # steal HBM bandwidth from the x/weight loads
            wdma = nc.sync.dma_start(
                wo[:], woT.ap().rearrange("(k p) t -> p k t", p=128))
            tile.add_dep_helper(vcopy.ins, wdma.ins, sync=False,
                                reason="defer wo prefetch past head proj")

            # ---- phase 2: attention, ACT-bound, PE slack filled with
            #      hp1 projections and the wo k0-3 partial pass ----
            with (
                tc.tile_pool(name="spsum", bufs=2, space="PSUM") as spsum,
                tc.tile_pool(name="opsum", bufs=1, space="PSUM") as opsum,
                tc.tile_pool(name="auxsum", bufs=2, space="PSUM") as auxsum,
            ):
                pid = nc.partition_id()
                rnk = pid & 3          # rank within the 4-core batch group
                qh2 = (pid >> 1) & 1   # which query-half holds my tokens
                colo = (pid & 1) * TSLICE

                # ---- filler work for hp0 attention slots: Q1/K1/V1 proj ----
                fill0 = []  # (cols, closure)
                _auxn = [0]

                def qk1_unit(wtile, dst, n):
                    def mmk(k, wtile=wtile, n=n, start=False, stop=False):
                        def fn(aux):
                            nc.tensor.matmul(
                                aux[:], wtile[:, k, 128:256],
                                xt[:, k, 512 * n:512 * (n + 1)],
                                start=(k == 0), stop=(k == NKD - 1))
                        return fn
                    def cpy(aux, dst=dst, n=n):
                        nc.vector.tensor_copy(
                            dst[:, 512 * n:512 * (n + 1)], aux[:])
                    return [mmk(k) for k in range(NKD)], cpy, [128, 512]

                def v_unit(mt, hlo):
                    fcol = 128 * (hlo // 2)
                    def mmk(k, mt=mt, fcol=fcol):
                        def fn(aux):
                            nc.tensor.matmul(
                                aux[:], xt[:, k, 128 * mt:128 * (mt + 1)],
                                wv[:, k, fcol:fcol + 128],
                                start=(k == 0), stop=(k == NKD - 1))
                        return fn
                    def cpy(aux, mt=mt, hlo=hlo):
                        nc.vector.tensor_copy(
                            vp[:, mt, hlo:hlo + 2, 0:64],
                            aux[:].rearrange("p (h d) -> p h d", h=2))
                    return [mmk(k) for k in range(NKD)], cpy, [128, 128]

                # V cols count double in the pump budget (ldweights-bound
                # 128-col matmuls)
                units = ([(v_unit(mt, 0), 2.0) for mt in range(NKT // 2, NKT)]
                         + [(qk1_unit(wk, kt[1], n), 1.0) for n in range(4)]
                         + [(qk1_unit(wq, qt[1], 0), 1.0)]
                         + [(v_unit(mt, 2), 2.0) for mt in range(NKT)]
                         + [(qk1_unit(wq, qt[1], n), 1.0) for n in range(1, 4)])
                for (mms, cpy, shape), wgt in units:
                    aux_box = []
                    def alloc(aux_box=aux_box, shape=shape):
                        aux_box.append(auxsum.tile(shape, F32, name=f"aux{_auxn[0]}", tag="aux"))
                        _auxn[0] += 1
                    cols = shape[1] * wgt
                    for i, mm in enumerate(mms):
                        def step(mm=mm, aux_box=aux_box, alloc=alloc, first=(i == 0)):
                            if first:
                                alloc()
                            mm(aux_box[-1])
                        fill0.append((cols, step))
                    fill0.append((0, lambda cpy=cpy, aux_box=aux_box:
                                  cpy(aux_box[-1])))

                # ---- filler work for hp1 attention slots: wo k0-3 pass ----
                rhs = [rhs_pool.tile([128, TSLICE], BF16, name=f"rhs{k}",
                                     tag=f"rhs{k}") for k in range(NKD)]
                ya = [ya_pool.tile([128, TSLICE], F32, name=f"ya{m}",
                                   tag=f"ya{m}") for m in range(NKD)]
                fill1 = []

                def woa_unit(m):
                    aux_box = []
                    def step(k, m=m, aux_box=aux_box):
                        def fn():
                            if k == 0:
                                aux_box.append(
                                    auxsum.tile([128, TSLICE], F32, name=f"auxw{m}", tag="aux"))
                            nc.tensor.matmul(
                                aux_box[-1][:], wo[:, k, 128 * m:128 * (m + 1)],
                                rhs[k][:], start=(k == 0), stop=(k == 3))
                        return fn
                    def cpy(m=m, aux_box=aux_box):
                        nc.vector.tensor_copy(ya[m][:], aux_box[-1][:])
                    return [step(k) for k in range(4)], cpy

                for m in range(NKD):
                    mms, cpy = woa_unit(m)
                    for mm in mms:
                        fill1.append((512, mm))
                    fill1.append((0, cpy))

                fstate = {"i0": 0, "c0": 0.0, "i1": 0, "c1": 0.0}

                def pump0(target):
                    while fstate["i0"] < len(fill0) and fstate["c0"] < target:
                        cols, fn = fill0[fstate["i0"]]
                        fstate["i0"] += 1
                        fn()
                        fstate["c0"] += cols

                def pump1(target):
                    while fstate["i1"] < len(fill1) and fstate["c1"] < target:
                        cols, fn = fill1[fstate["i1"]]
                        fstate["i1"] += 1
                        fn()
                        fstate["c1"] += cols

                WOA_START = 96   # slot where the wo k0-3 pass may begin

                def pump(slot):
                    if slot < 13:
                        # front-load V rows 8-15 while ACT is still filling
                        pump0(1400.0 * (slot + 1))
                    else:
                        pump0(18200.0 + 880.0 * (slot - 12))
                        if slot >= WOA_START:
                            pump1(768.0 * (slot - WOA_START + 1))

                # ---- attention slot loop ----
                pend = {}

                def add_pend(slot, fn):
                    pend.setdefault(slot, []).append(fn)

                def make_oslot(km, opA, opB, pts, hA, hB):
                    def fn():
                        nc.tensor.matmul(
                            opA[:], vp[:, km, hA, :], pts[km][:, 0:512],
                            start=(km == 0), stop=(km == NKT - 1))
                        nc.tensor.matmul(
                            opB[:], vp[:, km, hB, :], pts[km][:, 512:1024],
                            start=(km == 0), stop=(km == NKT - 1))
                    return fn

                def make_finalize(hp, q4, opA, opB):
                    def fn():
                        # evacuate psum on two engines in parallel
                        ocA = oc_pool.tile([128, 512], F32, tag="ocA")
                        ocB = oc_pool.tile([128, 512], F32, tag="ocB")
                        nc.vector.tensor_copy(ocA[:], opA[:])
                        nc.scalar.copy(ocB[:], opB[:])
                        # stack both heads' denominators -> one reciprocal
                        da = rb_pool.tile([128, 512], F32, tag="da")
                        nc.vector.stream_shuffle(
                            da[0:64, :], ocA[64:128, :], IDENT32)
                        nc.vector.tensor_copy(da[64:128, :], ocB[64:128, :])
                        rr = rb_pool.tile([128, 512], F32, tag="rr")
                        nc.vector.reciprocal(rr[:], da[:])
                        onA = on_pool.tile([64, 512], BF16, tag="onA")
                        onB = on_pool.tile([64, 512], BF16, tag="onB")
                        nc.vector.tensor_mul(onA[:], ocA[0:64, :], rr[0:64, :])
                        rlB = rb_pool.tile([64, 512], F32, tag="rlB")
                        nc.vector.stream_shuffle(rlB[:], rr[64:128, :], IDENT32)
                        nc.vector.tensor_mul(onB[:], ocB[0:64, :], rlB[:])
                        if hp == 0:
                            qh, co = q4 // 2, (q4 % 2) * TSLICE
                            nc.sync.dma_start(
                                agin0[qh, 0:64, co:co + TSLICE], onA[:])
                            nc.sync.dma_start(
                                agin0[qh, 64:128, co:co + TSLICE], onB[:])
                            if q4 % 2 == 1:
                                nc.gpsimd.collective_compute(
                                    "AllGather", mybir.AluOpType.bypass,
                                    replica_groups=rgroups,
                                    ins=[agin0[qh].opt()],
                                    outs=[agout0[qh:qh + 1, :, :, :].opt()],
                                )
                        else:
                            nc.sync.dma_start(agin1[q4, 0:64, :], onA[:])
                            nc.sync.dma_start(agin1[q4, 64:128, :], onB[:])
                            nc.gpsimd.collective_compute(
                                "AllGather", mybir.AluOpType.bypass,
                                replica_groups=rgroups,
                                ins=[agin1[q4].opt()],
                                outs=[agout1[q4:q4 + 1, :, :, :].opt()],
                            )
                    return fn

                slot = 0
                for hp in range(NHP):
                    hA, hB = 2 * hp, 2 * hp + 1
                    for q4 in range(NQ4):
                        c = hp * NQ4 + q4
                        base = 16 * c
                        opA = opsum.tile([128, 512], F32, tag="opA")
                        opB = opsum.tile([128, 512], F32, tag="opB")
                        pts = {}
                        oslot = [make_oslot(km, opA, opB, pts, hA, hB)
                                 for km in range(NKT)]
                        for km in range(NKT - 2):
                            add_pend(base + km + 3, oslot[km])
                        fin = make_finalize(hp, q4, opA, opB)
                        add_pend(base + 17,
                                 lambda a=oslot[14], b=oslot[15], f=fin:
                                 (a(), b(), f()))
                        if c == 3:
                            # prefetch wo rhs for hp0 once its gathers are done
                            add_pend(base + 17, lambda: [
                                nc.sync.dma_start(
                                    rhs[k][:],
                                    agout0[bass.ds(qh2, 1), k % GROUP, :,
                                           bass.ds(colo, TSLICE)].squeeze(0))
                                for k in range(GROUP)])

                        qs = slice(512 * q4, 512 * (q4 + 1))
                        for km in range(NKT):
                            ks = slice(128 * km, 128 * (km + 1))
                            sp = spsum.tile([128, 1024], F32, tag="sp")
                            nc.tensor.matmul(
                                sp[:, 0:512], kt[hp][0:64, ks],
                                qt[hp][0:64, qs], start=True, stop=True)
                            nc.tensor.matmul(
                                sp[:, 512:1024], kt[hp][64:128, ks],
                                qt[hp][64:128, qs], start=True, stop=True)
                            pt = pt_pool.tile([128, 1024], BF16, tag="pt")
                            nc.scalar.activation(pt[:], sp[:], AF.Exp,
                                                 scale=0.03125)
                            pts[km] = pt
                            for fn in pend.pop(slot, []):
                                fn()
                            pump(slot)
                            slot += 1

                # flush trailing scheduled items (last chunk's O + finalize)
                for s in sorted(pend):
                    for fn in pend.pop(s):
                        fn()
                pump0(1e18)
                pump1(1e18)

                # ---- tail: rhs for hp1, wo k4-7 + add partials, output ----
                for k in range(GROUP, NKD):
                    nc.sync.dma_start(
                        rhs[k][:],
                        agout1[bass.ds(rnk, 1), k % GROUP, :, :].squeeze(0))
                for m in range(NKD):
                    yb = auxsum.tile([128, TSLICE], F32, name=f"yb{m}", tag="aux")
                    for k in range(GROUP, NKD):
                        nc.tensor.matmul(
                            yb[:], wo[:, k, 128 * m:128 * (m + 1)], rhs[k][:],
                            start=(k == GROUP), stop=(k == NKD - 1))
                    yt_s = yt_pool.tile([128, TSLICE], BF16, tag="yt")
                    nc.vector.tensor_add(yt_s[:], yb[:], ya[m][:])
                    nc.sync.dma_start(yTb.ap()[128 * m:128 * (m + 1), :], yt_s[:])

    nc.compile()
    return nc


def _get_nc():
    if "nc" not in _CACHE:
        _CACHE["nc"] = _build()
    return _CACHE["nc"]


def _make_in_maps(x, Wq, Wk, Wv, Wo):
    # Wo rows permuted to match the AllToAll assembly order:
    # row i = (pair hp, source rank s, head-in-pair a, dim d) -> head 4s+2hp+a
    idx = np.arange(KD)
    hp, rem = idx // (GROUP * 128), idx % (GROUP * 128)
    s, r = rem // 128, rem % 128
    a, d = r // HS, r % HS
    perm = (GROUP * s + 2 * hp + a) * HS + d
    woTp = np.ascontiguousarray(Wo.T[perm]).astype(ml_dtypes.bfloat16)

    in_maps = []
    for c in range(NCORES):
        g, r = c // GROUP, c % GROUP
        rows = slice(r * HFEAT, (r + 1) * HFEAT)
        in_maps.append({
            "xT": np.ascontiguousarray(x[g].T).astype(ml_dtypes.bfloat16),
            "wqT": np.ascontiguousarray(Wq[rows].T).astype(ml_dtypes.bfloat16),
            "wkT": np.ascontiguousarray(Wk[rows].T).astype(ml_dtypes.bfloat16),
            "wvT": np.ascontiguousarray(Wv[rows].T).astype(ml_dtypes.bfloat16),
            "woT": woTp,
        })
    return in_maps


def kernel(x, Wq, Wk, Wv, Wo):
    from concourse import bass_utils

    x = np.asarray(x, dtype=np.float32)
    Wq = np.asarray(Wq, dtype=np.float32)
    Wk = np.asarray(Wk, dtype=np.float32)
    Wv = np.asarray(Wv, dtype=np.float32)
    Wo = np.asarray(Wo, dtype=np.float32)

    nc = _get_nc()
    in_maps = _make_in_maps(x, Wq, Wk, Wv, Wo)
    res = bass_utils.run_bass_kernel_spmd(nc, in_maps, core_ids=list(range(NCORES)))

    out = np.empty((B, T, KD), dtype=np.float32)
    for c in range(NCORES):
        g, r = c // GROUP, c % GROUP
        out[g, r * TSLICE:(r + 1) * TSLICE, :] = \
            res.results[c]["yTb"].astype(np.float32).T
    return out


# revision 41
# speedup vs baseline: 1.0616x; 1.0139x over previous
"""Multi-head attention (b=2, t=2048, k=1024, 16 heads) on 8 TRN2 NeuronCores.

Sharding: batch across 2 groups of 4 cores; within a group, heads are
tensor-parallel (4 heads/core = 2 head-pairs, full T).  Per-core pipeline:

  head:    Q/K proj for head-pair 0 (k-outer, 8 psum banks) + V for hp0.
  phase 2: per (hp, q4-chunk, km): S^T matmuls -> exp [128,1024] on ACT.
           O matmuls use V tiles with 64 ones-columns appended, so each O
           accumulation also produces the softmax denominators replicated in
           psum rows 64-127 (no separate denominator matmuls).  The ACT
           engine is the bottleneck; PE slack is filled with interleaved
           hp1 projections (during hp0 attention) and the Wo k0-3 partial
           pass (during hp1 attention, after hp0's AllToAll).
           Normalize: evacuate op psum (DVE+ACT copies), reciprocal_approx,
           stream_shuffle partition realign, multiply, DMA to DRAM.
  comms:   one 4-core AllToAll per head-pair (each peer gets only its own
           512 token columns) instead of AllGather.
  tail:    Wo k4-7 matmuls + add to k0-3 partials -> y^T slice (bf16).
"""

import sys

if '/opt/trn_rl_repo' not in sys.path:
    sys.path.insert(0, '/opt/trn_rl_repo')

import ml_dtypes
import numpy as np

B = 2
T = 2048
KD = 1024
NH = 16
HS = 64
NCORES = 8
GROUP = 4                 # cores per batch group
NH_LOC = NH // GROUP      # heads per core
NHP = NH_LOC // 2         # head-pairs per core
TSLICE = T // GROUP       # output tokens per core
HFEAT = NH_LOC * HS       # 256 local head features
NKT = T // 128            # 16 key-token tiles
NKD = KD // 128           # 8 model-feature tiles
NQ4 = T // 512            # 4 query chunks of 512

_CACHE = {}


def _build():
    import concourse.bass as bass
    import concourse.mybir as mybir
    import concourse.tile as tile
    from concourse import bacc

    F32 = mybir.dt.float32
    BF16 = mybir.dt.bfloat16
    AF = mybir.ActivationFunctionType
    IDENT32 = list(range(32))

    nc = bacc.Bacc("TRN2", target_bir_lowering=False, debug=False,
                   num_devices=NCORES)

    xT = nc.dram_tensor("xT", [KD, T], BF16, kind="ExternalInput")
    wqT = nc.dram_tensor("wqT", [KD, HFEAT], BF16, kind="ExternalInput")
    wkT = nc.dram_tensor("wkT", [KD, HFEAT], BF16, kind="ExternalInput")
    wvT = nc.dram_tensor("wvT", [KD, HFEAT], BF16, kind="ExternalInput")
    woT = nc.dram_tensor("woT", [KD, KD], BF16, kind="ExternalInput")
    yTb = nc.dram_tensor("yTb", [KD, TSLICE], BF16, kind="ExternalOutput")

    rgroups = [list(range(GROUP)), list(range(GROUP, 2 * GROUP))]

    with tile.TileContext(nc) as tc:
        with (
            tc.tile_pool(name="qk", bufs=1) as qk_pool,
            tc.tile_pool(name="vpp", bufs=1) as vp_pool,
            tc.tile_pool(name="xw", bufs=1) as xw_pool,
            tc.tile_pool(name="wop", bufs=1) as wo_pool,
            tc.tile_pool(name="pt", bufs=7) as pt_pool,
            tc.tile_pool(name="oc", bufs=2) as oc_pool,
            tc.tile_pool(name="rb", bufs=2) as rb_pool,
            tc.tile_pool(name="on", bufs=2) as on_pool,
            tc.tile_pool(name="rhs", bufs=1) as rhs_pool,
            tc.tile_pool(name="ya", bufs=1) as ya_pool,
            tc.tile_pool(name="yt", bufs=2) as yt_pool,
            tc.tile_pool(name="dram", bufs=1, space="DRAM") as dram,
        ):
            qt = [qk_pool.tile([128, T], BF16, name=f"qt{m}", tag=f"qt{m}")
                  for m in range(2)]
            kt = [qk_pool.tile([128, T], BF16, name=f"kt{m}", tag=f"kt{m}")
                  for m in range(2)]
            # V in bf16 token-major; per head 128 lhs columns: 0-63 = V dims,
            # 64-127 = 1.0 so the O matmul also emits denominator rows.
            vp = vp_pool.tile([128, NKT, NH_LOC, 128], BF16)
            nc.vector.memset(vp[:], 1.0)

            wq = xw_pool.tile([128, NKD, HFEAT], BF16)
            wk = xw_pool.tile([128, NKD, HFEAT], BF16)
            wv = xw_pool.tile([128, NKD, HFEAT], BF16)
            xt = xw_pool.tile([128, NKD, T], BF16)
            wo = wo_pool.tile([128, NKD, KD], BF16)

            # hp0 gathers at query-half granularity, hp1 at q4 granularity
            # (small final op keeps the collective off the critical tail)
            agin0 = dram.tile([2, 128, 2 * TSLICE], BF16, name="agin0")
            agout0 = dram.tile([2, GROUP, 128, 2 * TSLICE], BF16, name="agout0")
            agin1 = dram.tile([NQ4, 128, TSLICE], BF16, name="agin1")
            agout1 = dram.tile([NQ4, GROUP, 128, TSLICE], BF16, name="agout1")

            # x slices stream on the SP queue; batched weight loads ride the
            # ACT queue so they don't serialize behind the x stream
            for k in range(NKD):
                nc.sync.dma_start(xt[:, k, :], xT.ap()[128 * k:128 * (k + 1), :])
            for wtile, wdram in ((wq, wqT), (wk, wkT), (wv, wvT)):
                nc.scalar.dma_start(
                    wtile[:], wdram.ap().rearrange("(k p) t -> p k t", p=128))



            # ---- head: Q/K proj for hp0 (m=0), then V for hp0 ----
            with tc.tile_pool(name="hpsum", bufs=1, space="PSUM") as hpsum:
                acc = [hpsum.tile([128, 512], F32, name=f"acc{i}", tag=f"acc{i}")
                       for i in range(8)]
                for k in range(NKD):
                    for n in range(4):
                        nc.tensor.matmul(
                            acc[n][:], wq[:, k, 0:128],
                            xt[:, k, 512 * n:512 * (n + 1)],
                            start=(k == 0), stop=(k == NKD - 1))
                    for n in range(4):
                        nc.tensor.matmul(
                            acc[4 + n][:], wk[:, k, 0:128],
                            xt[:, k, 512 * n:512 * (n + 1)],
                            start=(k == 0), stop=(k == NKD - 1))
                for n in range(4):
                    nc.vector.tensor_copy(
                        qt[0][:, 512 * n:512 * (n + 1)], acc[n][:])
                    nc.vector.tensor_copy(
                        kt[0][:, 512 * n:512 * (n + 1)], acc[4 + n][:])

                vcopy = None
                for mt in range(NKT // 2):
                    ps = hpsum.tile([128, 128], F32, tag=f"acc{mt % 8}")
                    for k in range(NKD):
                        nc.tensor.matmul(
                            ps[:], xt[:, k, 128 * mt:128 * (mt + 1)],
                            wv[:, k, 0:128],
                            start=(k == 0), stop=(k == NKD - 1))
                    vcopy = nc.vector.tensor_copy(
                        vp[:, mt, 0:2, 0:64],
                        ps[:].rearrange("p (h d) -> p h d", h=2))

            # wo prefetch, deferred until the head V copies so it doesn't
            # steal HBM bandwidth from the x/weight loads
            wdma = nc.sync.dma_start(
                wo[:], woT.ap().rearrange("(k p) t -> p k t", p=128))
            tile.add_dep_helper(vcopy.ins, wdma.ins, sync=False,
                                reason="defer wo prefetch past head proj")

            # ---- phase 2: attention, ACT-bound, PE slack filled with
            #      hp1 projections and the wo k0-3 partial pass ----
            with (
                tc.tile_pool(name="spsum", bufs=2, space="PSUM") as spsum,
                tc.tile_pool(name="opsum", bufs=1, space="PSUM") as opsum,
                tc.tile_pool(name="auxsum", bufs=2, space="PSUM") as auxsum,
            ):
                pid = nc.partition_id()
                rnk = pid & 3          # rank within the 4-core batch group
                qh2 = (pid >> 1) & 1   # which query-half holds my tokens
                colo = (pid & 1) * TSLICE

                # ---- filler work for hp0 attention slots: Q1/K1/V1 proj ----
                fill0 = []  # (cols, closure)
                _auxn = [0]

                def qk1_unit(wtile, dst, n):
                    def mmk(k, wtile=wtile, n=n, start=False, stop=False):
                        def fn(aux):
                            nc.tensor.matmul(
                                aux[:], wtile[:, k, 128:256],
                                xt[:, k, 512 * n:512 * (n + 1)],
                                start=(k == 0), stop=(k == NKD - 1))
                        return fn
                    def cpy(aux, dst=dst, n=n):
                        nc.vector.tensor_copy(
                            dst[:, 512 * n:512 * (n + 1)], aux[:])
                    return [mmk(k) for k in range(NKD)], cpy, [128, 512]

                def v_unit(mt, hlo):
                    fcol = 128 * (hlo // 2)
                    def mmk(k, mt=mt, fcol=fcol):
                        def fn(aux):
                            nc.tensor.matmul(
                                aux[:], xt[:, k, 128 * mt:128 * (mt + 1)],
                                wv[:, k, fcol:fcol + 128],
                                start=(k == 0), stop=(k == NKD - 1))
                        return fn
                    def cpy(aux, mt=mt, hlo=hlo):
                        nc.vector.tensor_copy(
                            vp[:, mt, hlo:hlo + 2, 0:64],
                            aux[:].rearrange("p (h d) -> p h d", h=2))
                    return [mmk(k) for k in range(NKD)], cpy, [128, 128]

                # V cols count double in the pump budget (ldweights-bound
                # 128-col matmuls); order approximates earliest-deadline-first
                units = ([(v_unit(mt, 0), 2.0, f"v0.{mt}")
                          for mt in range(NKT // 2, NKT)]
                         + [(qk1_unit(wk, kt[1], n), 1.0, f"k1.{n}")
                            for n in range(4)]
                         + [(qk1_unit(wq, qt[1], 0), 1.0, "q1.0")]
                         + [(v_unit(mt, 2), 2.0, f"v1.{mt}") for mt in range(13)]
                         + [(qk1_unit(wq, qt[1], 1), 1.0, "q1.1")]
                         + [(v_unit(mt, 2), 2.0, f"v1.{mt}")
                            for mt in range(13, NKT)]
                         + [(qk1_unit(wq, qt[1], n), 1.0, f"q1.{n}")
                            for n in (2, 3)])
                fdone = set()
                for (mms, cpy, shape), wgt, label in units:
                    aux_box = []
                    def alloc(aux_box=aux_box, shape=shape):
                        aux_box.append(auxsum.tile(shape, F32, name=f"aux{_auxn[0]}", tag="aux"))
                        _auxn[0] += 1
                    cols = shape[1] * wgt
                    for i, mm in enumerate(mms):
                        def step(mm=mm, aux_box=aux_box, alloc=alloc, first=(i == 0)):
                            if first:
                                alloc()
                            mm(aux_box[-1])
                        fill0.append((cols, step, None))
                    fill0.append((0, lambda cpy=cpy, aux_box=aux_box:
                                  cpy(aux_box[-1]), label))

                rhs = [rhs_pool.tile([128, TSLICE], BF16, name=f"rhs{k}",
                                     tag=f"rhs{k}") for k in range(NKD)]

                fstate = {"i0": 0, "c0": 0.0}

                def pump_one():
                    cols, fn, label = fill0[fstate["i0"]]
                    fstate["i0"] += 1
                    fn()
                    fstate["c0"] += cols
                    if label is not None:
                        fdone.add(label)

                def pump0(target):
                    while fstate["i0"] < len(fill0) and fstate["c0"] < target:
                        pump_one()

                def ensure(label):
                    # correctness: the filler producing `label` must be
                    # emitted before the consumer instruction
                    while label not in fdone:
                        pump_one()

                def pump(slot):
                    if slot < 13:
                        # front-load V rows 8-15 while ACT is still filling
                        pump0(1400.0 * (slot + 1))
                    else:
                        pump0(18200.0 + 800.0 * (slot - 12))

                # ---- attention slot loop ----
                pend = {}

                def add_pend(slot, fn):
                    pend.setdefault(slot, []).append(fn)

                def make_oslot(km, opA, opB, pts, hA, hB, hp):
                    def fn():
                        if hp == 0 and km >= NKT // 2:
                            ensure(f"v0.{km}")
                        elif hp == 1:
                            ensure(f"v1.{km}")
                        nc.tensor.matmul(
                            opA[:], vp[:, km, hA, :], pts[km][:, 0:512],
                            start=(km == 0), stop=(km == NKT - 1))
                        nc.tensor.matmul(
                            opB[:], vp[:, km, hB, :], pts[km][:, 512:1024],
                            start=(km == 0), stop=(km == NKT - 1))
                    return fn

                def make_finalize(hp, q4, opA, opB):
                    def fn():
                        # evacuate psum on two engines in parallel
                        ocA = oc_pool.tile([128, 512], F32, tag="ocA")
                        ocB = oc_pool.tile([128, 512], F32, tag="ocB")
                        nc.vector.tensor_copy(ocA[:], opA[:])
                        nc.scalar.copy(ocB[:], opB[:])
                        # stack both heads' denominators -> one reciprocal
                        da = rb_pool.tile([128, 512], F32, tag="da")
                        nc.vector.stream_shuffle(
                            da[0:64, :], ocA[64:128, :], IDENT32)
                        nc.vector.tensor_copy(da[64:128, :], ocB[64:128, :])
                        rr = rb_pool.tile([128, 512], F32, tag="rr")
                        nc.vector.reciprocal(rr[:], da[:])
                        onA = on_pool.tile([64, 512], BF16, tag="onA")
                        onB = on_pool.tile([64, 512], BF16, tag="onB")
                        nc.vector.tensor_mul(onA[:], ocA[0:64, :], rr[0:64, :])
                        rlB = rb_pool.tile([64, 512], F32, tag="rlB")
                        nc.vector.stream_shuffle(rlB[:], rr[64:128, :], IDENT32)
                        nc.vector.tensor_mul(onB[:], ocB[0:64, :], rlB[:])
                        if hp == 0:
                            qh, co = q4 // 2, (q4 % 2) * TSLICE
                            nc.sync.dma_start(
                                agin0[qh, 0:64, co:co + TSLICE], onA[:])
                            nc.sync.dma_start(
                                agin0[qh, 64:128, co:co + TSLICE], onB[:])
                            if q4 % 2 == 1:
                                nc.gpsimd.collective_compute(
                                    "AllGather", mybir.AluOpType.bypass,
                                    replica_groups=rgroups,
                                    ins=[agin0[qh].opt()],
                                    outs=[agout0[qh:qh + 1, :, :, :].opt()],
                                )
                        else:
                            nc.sync.dma_start(agin1[q4, 0:64, :], onA[:])
                            nc.sync.dma_start(agin1[q4, 64:128, :], onB[:])
                            nc.gpsimd.collective_compute(
                                "AllGather", mybir.AluOpType.bypass,
                                replica_groups=rgroups,
                                ins=[agin1[q4].opt()],
                                outs=[agout1[q4:q4 + 1, :, :, :].opt()],
                            )
                    return fn

                slot = 0
                for hp in range(NHP):
                    hA, hB = 2 * hp, 2 * hp + 1
                    for q4 in range(NQ4):
                        c = hp * NQ4 + q4
                        base = 16 * c
                        opA = opsum.tile([128, 512], F32, tag="opA")
                        opB = opsum.tile([128, 512], F32, tag="opB")
                        pts = {}
                        oslot = [make_oslot(km, opA, opB, pts, hA, hB, hp)
                                 for km in range(NKT)]
                        for km in range(NKT - 2):
                            add_pend(base + km + 3, oslot[km])
                        fin = make_finalize(hp, q4, opA, opB)
                        add_pend(base + 17,
                                 lambda a=oslot[14], b=oslot[15], f=fin:
                                 (a(), b(), f()))
                        if hp == 1:
                            for n in range(4):
                                ensure(f"k1.{n}")
                            ensure(f"q1.{q4}")
                        qs = slice(512 * q4, 512 * (q4 + 1))
                        for km in range(NKT):
                            ks = slice(128 * km, 128 * (km + 1))
                            sp = spsum.tile([128, 1024], F32, tag="sp")
                            nc.tensor.matmul(
                                sp[:, 0:512], kt[hp][0:64, ks],
                                qt[hp][0:64, qs], start=True, stop=True)
                            nc.tensor.matmul(
                                sp[:, 512:1024], kt[hp][64:128, ks],
                                qt[hp][64:128, qs], start=True, stop=True)
                            pt = pt_pool.tile([128, 1024], BF16, tag="pt")
                            nc.scalar.activation(pt[:], sp[:], AF.Exp,
                                                 scale=0.03125)
                            pts[km] = pt
                            for fn in pend.pop(slot, []):
                                fn()
                            pump(slot)
                            slot += 1

                # flush trailing scheduled items (last chunk's O + finalize)
                for s in sorted(pend):
                    for fn in pend.pop(s):
                        fn()
                pump0(1e18)

                # ---- tail: rhs loads + full wo pass + output ----
                for k in range(NKD):
                    if k < GROUP:
                        src = agout0[bass.ds(qh2, 1), k, :,
                                     bass.ds(colo, TSLICE)].squeeze(0)
                    else:
                        src = agout1[bass.ds(rnk, 1), k - GROUP, :, :].squeeze(0)
                    nc.sync.dma_start(rhs[k][:], src)
                for m in range(NKD):
                    yb = auxsum.tile([128, TSLICE], F32, name=f"yb{m}", tag="aux")
                    for k in range(NKD):
                        nc.tensor.matmul(
                            yb[:], wo[:, k, 128 * m:128 * (m + 1)], rhs[k][:],
                            start=(k == 0), stop=(k == NKD - 1))
                    yt_s = yt_pool.tile([128, TSLICE], BF16, tag="yt")
                    nc.vector.tensor_copy(yt_s[:], yb[:])
                    nc.sync.dma_start(yTb.ap()[128 * m:128 * (m + 1), :], yt_s[:])

    nc.compile()
    return nc


def _get_nc():
    if "nc" not in _CACHE:
        _CACHE["nc"] = _build()
    return _CACHE["nc"]


def _make_in_maps(x, Wq, Wk, Wv, Wo):
    # Wo rows permuted to match the AllToAll assembly order:
    # row i = (pair hp, source rank s, head-in-pair a, dim d) -> head 4s+2hp+a
    idx = np.arange(KD)
    hp, rem = idx // (GROUP * 128), idx % (GROUP * 128)
    s, r = rem // 128, rem % 128
    a, d = r // HS, r % HS
    perm = (GROUP * s + 2 * hp + a) * HS + d
    woTp = np.ascontiguousarray(Wo.T[perm]).astype(ml_dtypes.bfloat16)

    in_maps = []
    for c in range(NCORES):
        g, r = c // GROUP, c % GROUP
        rows = slice(r * HFEAT, (r + 1) * HFEAT)
        in_maps.append({
            "xT": np.ascontiguousarray(x[g].T).astype(ml_dtypes.bfloat16),
            "wqT": np.ascontiguousarray(Wq[rows].T).astype(ml_dtypes.bfloat16),
            "wkT": np.ascontiguousarray(Wk[rows].T).astype(ml_dtypes.bfloat16),
            "wvT": np.ascontiguousarray(Wv[rows].T).astype(ml_dtypes.bfloat16),
            "woT": woTp,
        })
    return in_maps


def kernel(x, Wq, Wk, Wv, Wo):
    from concourse import bass_utils

    x = np.asarray(x, dtype=np.float32)
    Wq = np.asarray(Wq, dtype=np.float32)
    Wk = np.asarray(Wk, dtype=np.float32)
    Wv = np.asarray(Wv, dtype=np.float32)
    Wo = np.asarray(Wo, dtype=np.float32)

    nc = _get_nc()
    in_maps = _make_in_maps(x, Wq, Wk, Wv, Wo)
    res = bass_utils.run_bass_kernel_spmd(nc, in_maps, core_ids=list(range(NCORES)))

    out = np.empty((B, T, KD), dtype=np.float32)
    for c in range(NCORES):
        g, r = c // GROUP, c % GROUP
        out[g, r * TSLICE:(r + 1) * TSLICE, :] = \
            res.results[c]["yTb"].astype(np.float32).T
    return out
